# revision 15
# baseline (speedup 1.0000x reference)
"""Trainium2 Bass kernel for nn_DecoderFusionBlock (VSS/Mamba decoder fusion block).

Two-pass SPMD over 8 cores (collectives unavailable under this runtime):
  pass 1: core c -> batch b=c//2, plane=c%2 (row-/col-major spatial order).
          Runs proj/LN/in_proj/dwconv/silu, then the selective scan for the
          plane's two directions (forward + reversed via reversed access
          patterns), producing the plane's merge partial Q (already rotated
          to row-major via data-driven masks), plus x (residual) and z (gate).
  host:   ym[b] = Q[2b] + Q[2b+1]  (the only cross-core reduction)
  pass 2: core c -> batch b=c//2: out-norm, gate, out_proj+residual,
          ConvBlock, final LN.
"""

import contextlib
import numpy as np

import concourse.bass as bass
import concourse.tile as tile
from concourse import bacc, mybir
from concourse.bass_utils import run_bass_kernel_spmd

f32 = mybir.dt.float32
f32r = mybir.dt.float32r
AF = mybir.ActivationFunctionType
OP_ = mybir.AluOpType

B_, H_, W_ = 4, 48, 48
L = H_ * W_
CIN, COUT = 192, 96
DIN, NST, RNK, KDIR = 192, 16, 6, 4
HID = 192
LC = 256
NCH = L // LC
NG = 4                           # n-values per scan group (4 groups of 4)
MMC = 512
EPS = 1e-5
DT0, DT1 = 128, 64
MM = [(s, min(MMC, L - s)) for s in range(0, L, MMC)]


def _fc(ap, c, lc=LC):
    return ap[:, c * lc:(c + 1) * lc]


def _rc(ap, c, lc=LC):
    hi = L - c * lc - 1
    lo = L - (c + 1) * lc - 1
    return ap[:, hi::-1] if lo < 0 else ap[:, hi:lo:-1]


def _swap_free(a):
    return bass.AP(tensor=a.tensor, offset=a.offset, ap=[a.ap[0], a.ap[2], a.ap[1]])


def _rep(a, n):
    return bass.AP(tensor=a.tensor, offset=a.offset, ap=[a.ap[0], [0, n], a.ap[1]])


def _twh(a):
    st = a.ap[1][0]
    return bass.AP(tensor=a.tensor, offset=a.offset,
                   ap=[a.ap[0], [st, 48], [48 * st, 48]])


def _pl3(a):
    st = a.ap[1][0]
    return bass.AP(tensor=a.tensor, offset=a.offset,
                   ap=[a.ap[0], [48 * st, 48], [st, 48]])


def _r(ap):
    # plain fp32 matmuls: the BIR verifier requires explicit rounding ops for
    # f32r operands, which would cost more than the 4x PE slowdown saves here
    return ap


# ---------------------------------------------------------------- pass 1
def build_nc1():
    nc = bacc.Bacc("TRN2", target_bir_lowering=False, debug=False, num_devices=8)
    din = {}

    def I(name, shape):
        din[name] = nc.dram_tensor(name, shape, f32, kind="ExternalInput")

    I("xc_t", [CIN, L]); I("projW", [CIN, COUT]); I("projb", [COUT, 1])
    I("W1", [COUT, 2 * DIN]); I("b1", [2 * DIN, 1])
    I("convW", [DIN, 9]); I("convb", [DIN, 1])
    I("xpw", [2, DIN, RNK + 2 * NST]); I("dtw", [2, RNK, DIN])
    I("dtb", [2, DIN, 1]); I("acoef", [2, DIN, NST]); I("dvec", [2, DIN, 1])
    I("sel16", [96, NST * 128]); I("mrow", [DIN, 1]); I("mcol", [DIN, 1])
    oq_d = nc.dram_tensor("oq", [DIN, L], f32, kind="ExternalOutput")
    ox_d = nc.dram_tensor("ox", [COUT, L], f32, kind="ExternalOutput")
    oz_d = nc.dram_tensor("oz", [DIN, L], f32, kind="ExternalOutput")

    ctx = contextlib.ExitStack()
    with tile.TileContext(nc) as tc, ctx:
        const = ctx.enter_context(tc.tile_pool(name="const", bufs=1))
        big = ctx.enter_context(tc.tile_pool(name="big", bufs=1))
        work = ctx.enter_context(tc.tile_pool(name="work", bufs=2))
        scn = ctx.enter_context(tc.tile_pool(name="scn", bufs=1))
        psM = ctx.enter_context(tc.tile_pool(name="psM", bufs=2, space="PSUM"))
        psB = ctx.enter_context(tc.tile_pool(name="psB", bufs=1, space="PSUM"))

        def load2(name, rows, cols):
            t0 = const.tile([DT0, cols], f32, tag=name + "0", name=name + "0")
            t1 = const.tile([DT1, cols], f32, tag=name + "1", name=name + "1")
            nc.sync.dma_start(t0[:], din[name][0:DT0])
            nc.sync.dma_start(t1[:], din[name][DT0:rows])
            return t0, t1

        projW0, projW1 = load2("projW", CIN, COUT)
        projb = const.tile([COUT, 1], f32)
        nc.sync.dma_start(projb[:], din["projb"][:])
        W1t = const.tile([COUT, 2 * DIN], f32)
        nc.sync.dma_start(W1t[:], din["W1"][:])
        b1x0 = const.tile([DT0, 1], f32); nc.sync.dma_start(b1x0[:], din["b1"][0:128])
        b1x1 = const.tile([DT1, 1], f32); nc.sync.dma_start(b1x1[:], din["b1"][128:192])
        b1z0 = const.tile([DT0, 1], f32); nc.sync.dma_start(b1z0[:], din["b1"][192:320])
        b1z1 = const.tile([DT1, 1], f32); nc.sync.dma_start(b1z1[:], din["b1"][320:384])
        convW0, convW1 = load2("convW", DIN, 9)
        convb0, convb1 = load2("convb", DIN, 1)
        sel16 = const.tile([96, NST * 128], f32)
        nc.sync.dma_start(sel16[:], din["sel16"][:])
        mrow0, mrow1 = load2("mrow", DIN, 1)
        mcol0, mcol1 = load2("mcol", DIN, 1)
        kw = []
        for k in range(2):
            xp0 = const.tile([DT0, RNK + 2 * NST], f32, name=f"xp{k}0")
            xp1 = const.tile([DT1, RNK + 2 * NST], f32, name=f"xp{k}1")
            nc.sync.dma_start(xp0[:], din["xpw"][k, 0:DT0])
            nc.sync.dma_start(xp1[:], din["xpw"][k, DT0:DIN])
            dtw = const.tile([RNK, DIN], f32, name=f"dtw{k}")
            nc.sync.dma_start(dtw[:], din["dtw"][k])
            dtb0 = const.tile([DT0, 1], f32, name=f"dtb{k}0")
            dtb1 = const.tile([DT1, 1], f32, name=f"dtb{k}1")
            nc.sync.dma_start(dtb0[:], din["dtb"][k, 0:DT0])
            nc.sync.dma_start(dtb1[:], din["dtb"][k, DT0:DIN])
            ac0 = const.tile([DT0, NST], f32, name=f"ac{k}0")
            ac1 = const.tile([DT1, NST], f32, name=f"ac{k}1")
            nc.sync.dma_start(ac0[:], din["acoef"][k, 0:DT0])
            nc.sync.dma_start(ac1[:], din["acoef"][k, DT0:DIN])
            dv0 = const.tile([DT0, 1], f32, name=f"dv{k}0")
            dv1 = const.tile([DT1, 1], f32, name=f"dv{k}1")
            nc.sync.dma_start(dv0[:], din["dvec"][k, 0:DT0])
            nc.sync.dma_start(dv1[:], din["dvec"][k, DT0:DIN])
            kw.append(dict(xp=(xp0, xp1), dtw=dtw, dtb=(dtb0, dtb1),
                           ac=(ac0, ac1), dv=(dv0, dv1)))

        ones128 = const.tile([128, 1], f32); nc.vector.memset(ones128[:], 1.0)
        onesrow = const.tile([1, 128], f32); nc.vector.memset(onesrow[:], 1.0)
        epsc = const.tile([1, 1], f32); nc.vector.memset(epsc[:], EPS)

        # ---- load + proj ----
        xc0 = big.tile([DT0, L], f32, tag="s0")
        xc1 = big.tile([DT1, L], f32, tag="s1")
        nc.sync.dma_start(xc0[:], din["xc_t"][0:DT0])
        nc.sync.dma_start(xc1[:], din["xc_t"][DT0:CIN])
        x_t = big.tile([COUT, L], f32, tag="s2")
        for (s, w) in MM:
            ps = psM.tile([128, MMC], f32, tag="mm", name="psproj")
            nc.tensor.matmul(ps[:COUT, :w], _r(projW0[:]), _r(xc0[:, s:s + w]),
                             start=True, stop=False)
            nc.tensor.matmul(ps[:COUT, :w], _r(projW1[:]), _r(xc1[:, s:s + w]),
                             start=False, stop=True)
            nc.scalar.activation(x_t[:, s:s + w], ps[:COUT, :w], AF.Identity,
                                 bias=projb[:])
        nc.sync.dma_start(ox_d[:], x_t[:])

        # ---- LN1 (over 96 channel partitions), fused stats+apply per chunk ----
        xn_t = big.tile([COUT, L], f32, tag="s0b")
        for (s, w) in MM:
            ps = psM.tile([128, MMC], f32, tag="mm", name="pss1")
            nc.tensor.matmul(ps[:1, :w], _r(ones128[:COUT]), _r(x_t[:, s:s + w]),
                             start=True, stop=True)
            mrw = work.tile([1, MMC], f32, tag="mrw", bufs=1)
            nc.scalar.activation(mrw[:, :w], ps[:1, :w], AF.Copy, scale=1.0 / COUT)
            sq = work.tile([128, MMC], f32, tag="sqc", bufs=1)
            nc.vector.tensor_tensor(out=sq[:COUT, :w], in0=x_t[:, s:s + w],
                                    in1=x_t[:, s:s + w], op=OP_.mult)
            ps2 = psM.tile([128, MMC], f32, tag="mm", name="pss2")
            nc.tensor.matmul(ps2[:1, :w], _r(ones128[:COUT]), _r(sq[:COUT, :w]),
                             start=True, stop=True)
            mq = work.tile([1, MMC], f32, tag="mq", bufs=1)
            nc.scalar.activation(mq[:, :w], ps2[:1, :w], AF.Copy, scale=1.0 / COUT)
            msq = work.tile([1, MMC], f32, tag="msq", bufs=1)
            nc.vector.tensor_tensor(out=msq[:, :w], in0=mrw[:, :w],
                                    in1=mrw[:, :w], op=OP_.mult)
            nc.vector.tensor_tensor(out=mq[:, :w], in0=mq[:, :w],
                                    in1=msq[:, :w], op=OP_.subtract)
            nc.scalar.activation(mq[:, :w], mq[:, :w], AF.Sqrt, bias=epsc[:])
            rsw = work.tile([1, MMC], f32, tag="rsw", bufs=1)
            nc.vector.reciprocal(rsw[:, :w], mq[:, :w])
            pm = psM.tile([128, MMC], f32, tag="mm", name="psbm")
            nc.tensor.matmul(pm[:, :w], _r(onesrow[:]), _r(mrw[:, :w]),
                             start=True, stop=True)
            pr = psM.tile([128, MMC], f32, tag="mm", name="psbr")
            nc.tensor.matmul(pr[:, :w], _r(onesrow[:]), _r(rsw[:, :w]),
                             start=True, stop=True)
            nc.vector.tensor_tensor(out=xn_t[:, s:s + w], in0=x_t[:, s:s + w],
                                    in1=pm[:COUT, :w], op=OP_.subtract)
            nc.vector.tensor_tensor(out=xn_t[:, s:s + w], in0=xn_t[:, s:s + w],
                                    in1=pr[:COUT, :w], op=OP_.mult)

        # ---- in_proj (x-part to xm tiles, z-part straight to DRAM) ----
        xm0 = big.tile([DT0, L], f32, tag="s3")
        xm1 = big.tile([DT1, L], f32, tag="s1b")
        for (s, w) in MM:
            for (coff, rows, bcol, dst, zoff) in (
                    (0, DT0, b1x0, xm0, None), (DT0, DT1, b1x1, xm1, None),
                    (DIN, DT0, b1z0, None, 0), (DIN + DT0, DT1, b1z1, None, DT0)):
                ps = psM.tile([128, MMC], f32, tag="mm", name="psip")
                nc.tensor.matmul(ps[:rows, :w], _r(W1t[:, coff:coff + rows]),
                                 _r(xn_t[:, s:s + w]), start=True, stop=True)
                if dst is not None:
                    nc.scalar.activation(dst[:, s:s + w], ps[:rows, :w], AF.Identity,
                                         bias=bcol[:])
                else:
                    zc = work.tile([128, MMC], f32, tag="zc", bufs=1)
                    nc.scalar.activation(zc[:rows, :w], ps[:rows, :w], AF.Identity,
                                         bias=bcol[:])
                    nc.sync.dma_start(oz_d[zoff:zoff + rows, s:s + w], zc[:rows, :w])

        # ---- depthwise conv + silu ----
        cv0 = big.tile([DT0, L], f32, tag="s2b")
        cv1 = big.tile([DT1, L], f32, tag="s4")
        for (src, wt, rows, out, eng) in ((xm0, convW0, DT0, cv0, nc.vector),
                                          (xm1, convW1, DT1, cv1, nc.gpsimd)):
            pad = work.tile([128, 50, 50], f32, tag="pad", bufs=1)
            eng.memset(pad[:rows], 0.0)
            eng.tensor_copy(out=pad[:rows, 1:49, 1:49], in_=_pl3(src[:]))
            ov = _pl3(out[:])
            for j in range(9):
                dy, dx = divmod(j, 3)
                view = pad[:rows, dy:dy + 48, dx:dx + 48]
                if j == 0:
                    nc.vector.tensor_scalar_mul(ov, view, wt[:, 0:1])
                else:
                    nc.vector.scalar_tensor_tensor(out=ov, in0=view, scalar=wt[:, j:j + 1],
                                                   in1=ov, op0=OP_.mult, op1=OP_.add)
        xs0 = big.tile([DT0, L], f32, tag="s3b")
        xs1 = big.tile([DT1, L], f32, tag="s5")
        nc.scalar.activation(xs0[:], cv0[:], AF.Silu, bias=convb0[:])
        nc.scalar.activation(xs1[:], cv1[:], AF.Silu, bias=convb1[:])
        # transposed plane, then data-driven select (both in place into xs)
        xt0 = big.tile([DT0, L], f32, tag="s2b", name="xt0")
        xt1 = big.tile([DT1, L], f32, tag="s4", name="xt1")
        nc.vector.tensor_copy(out=xt0[:], in_=_twh(xs0[:]))
        nc.gpsimd.tensor_copy(out=xt1[:], in_=_twh(xs1[:]))
        nc.vector.tensor_scalar_mul(xs0[:], xs0[:], mrow0[:])
        nc.vector.scalar_tensor_tensor(out=xs0[:], in0=xt0[:], scalar=mcol0[:],
                                       in1=xs0[:], op0=OP_.mult, op1=OP_.add)
        nc.vector.tensor_scalar_mul(xs1[:], xs1[:], mrow1[:])
        nc.vector.scalar_tensor_tensor(out=xs1[:], in0=xt1[:], scalar=mcol1[:],
                                       in1=xs1[:], op0=OP_.mult, op1=OP_.add)

        # ---- scan: k=0 forward, k=1 reversed ----
        P0 = big.tile([DT0, L], f32, tag="s6")
        P1 = big.tile([DT1, L], f32, tag="s7")
        for k in range(2):
            rev = (k == 1)
            W = kw[k]
            U96 = big.tile([96, L], f32, tag="u96", name=f"u96_{k}")
            for (s, w) in MM:
                ps = psM.tile([128, MMC], f32, tag="mm", name="psU")
                for (coff, ubase, m) in ((0, 0, RNK), (RNK, 32, NST),
                                         (RNK + NST, 64, NST)):
                    nc.tensor.matmul(ps[ubase:ubase + m, :w],
                                     _r(W["xp"][0][:, coff:coff + m]),
                                     _r(xs0[:, s:s + w]), start=True, stop=False)
                    nc.tensor.matmul(ps[ubase:ubase + m, :w],
                                     _r(W["xp"][1][:, coff:coff + m]),
                                     _r(xs1[:, s:s + w]), start=False, stop=True)
                for (ubase, m) in ((0, RNK), (32, NST), (64, NST)):
                    nc.vector.tensor_copy(out=U96[ubase:ubase + m, s:s + w],
                                          in_=ps[ubase:ubase + m, :w])

            hp = [[scn.tile([128, NG, 1], f32, tag=f"hp{dt}{h}", name=f"hp{dt}{h}")
                   for h in range(NST // NG)] for dt in range(2)]
            for dt in range(2):
                for h in range(NST // NG):
                    nc.vector.memset(hp[dt][h][:], 0.0)

            for c in range(NCH):
                uslice = (_rc(U96[0:RNK, :], c) if rev else _fc(U96[0:RNK, :], c))
                bsl = (_rc(U96[32:32 + NST, :], c) if rev else _fc(U96[32:32 + NST, :], c))
                csl = (_rc(U96[64:64 + NST, :], c) if rev else _fc(U96[64:64 + NST, :], c))
                # delta / dx chunks for both dtiles
                dcs, dxs = [], []
                for dt, (rows, dtbc, eng) in enumerate(((DT0, W["dtb"][0], nc.vector),
                                                        (DT1, W["dtb"][1], nc.gpsimd))):
                    ps = psM.tile([128, LC], f32, tag="mm", name="psdt")
                    nc.tensor.matmul(ps[:rows, :], _r(W["dtw"][:, dt * DT0:dt * DT0 + rows]),
                                     _r(uslice), start=True, stop=True)
                    dc = work.tile([128, LC], f32, tag=f"dc{dt}", name=f"dc{dt}", bufs=1)
                    nc.scalar.activation(dc[:rows], ps[:rows, :], AF.Exp,
                                         bias=dtbc[:])
                    nc.scalar.activation(dc[:rows], dc[:rows], AF.Ln, bias=1.0)
                    xsc = _rc((xs0 if dt == 0 else xs1)[:], c) if rev \
                        else _fc((xs0 if dt == 0 else xs1)[:], c)
                    dxc = work.tile([128, LC], f32, tag=f"dxc{dt}", name=f"dxc{dt}", bufs=1)
                    eng.tensor_tensor(out=dxc[:rows], in0=dc[:rows], in1=xsc, op=OP_.mult)
                    dcs.append(dc); dxs.append(dxc)

                yhs = []
                for h in range(NST // NG):
                    bt = psB.tile([128, NG, LC], f32, tag="bb", name="bb")
                    ct = psB.tile([128, NG, LC], f32, tag="cb", name="cb")
                    for q in range(NG):
                        n = h * NG + q
                        slb = _r(sel16[32:48, n * 128:(n + 1) * 128])
                        slc = _r(sel16[64:80, n * 128:(n + 1) * 128])
                        nc.tensor.matmul(bt[:, q, :], slb, _r(bsl), start=True, stop=True)
                        nc.tensor.matmul(ct[:, q, :], slc, _r(csl), start=True, stop=True)
                    for dt, (rows, eng) in enumerate(((DT0, nc.vector), (DT1, nc.vector))):
                        dA = scn.tile([128, NG, LC + 1], f32, tag=f"dA{dt}",
                                      name=f"dA{dt}", bufs=2)
                        dBu = scn.tile([128, NG, LC + 1], f32, tag=f"dBu{dt}",
                                       name=f"dBu{dt}")
                        Ht = scn.tile([128, NG, LC + 1], f32, tag=f"H{dt}",
                                      name=f"H{dt}")
                        nc.vector.memset(dA[:rows, :, 0:1], 0.0)
                        nc.vector.tensor_copy(out=dBu[:rows, :, 0:1], in_=hp[dt][h][:rows])
                        for q in range(NG):
                            n = h * NG + q
                            nc.scalar.activation(dA[:rows, q, 1:], dcs[dt][:rows], AF.Exp,
                                                 scale=W["ac"][dt][:, n:n + 1])
                        nc.vector.tensor_tensor(out=dBu[:rows, :, 1:],
                                                in0=_rep(dxs[dt][:rows], NG),
                                                in1=bt[:rows], op=OP_.mult)
                        eng.tensor_tensor_scan(
                            out=Ht[:rows].rearrange("p a b -> p (a b)"),
                            data0=dA[:rows].rearrange("p a b -> p (a b)"),
                            data1=dBu[:rows].rearrange("p a b -> p (a b)"),
                            initial=0.0, op0=OP_.mult, op1=OP_.add)
                        nc.vector.tensor_copy(out=hp[dt][h][:rows],
                                              in_=Ht[:rows, :, LC:LC + 1])
                        # G = H * C, into dA's buffer
                        nc.vector.tensor_tensor(out=dA[:rows, :, 1:],
                                                in0=Ht[:rows, :, 1:],
                                                in1=ct[:rows], op=OP_.mult)
                        if h == 0:
                            yh = scn.tile([128, LC], f32, tag=f"yh{dt}",
                                          name=f"yh{dt}")
                            nc.vector.tensor_reduce(
                                out=yh[:rows], in_=_swap_free(dA[:rows, :, 1:]),
                                axis=mybir.AxisListType.X, op=OP_.add)
                            yhs.append(yh)
                        else:
                            yh2 = scn.tile([128, LC], f32, tag=f"yh2{dt}",
                                           name=f"yh2{dt}")
                            nc.vector.tensor_reduce(
                                out=yh2[:rows], in_=_swap_free(dA[:rows, :, 1:]),
                                axis=mybir.AxisListType.X, op=OP_.add)
                            nc.vector.tensor_tensor(out=yhs[dt][:rows],
                                                    in0=yhs[dt][:rows],
                                                    in1=yh2[:rows], op=OP_.add)
                # += D * xs ; accumulate into P
                for dt, (rows, Pt, xst, eng) in enumerate(
                        ((DT0, P0, xs0, nc.vector), (DT1, P1, xs1, nc.gpsimd))):
                    xsc = _rc(xst[:], c) if rev else _fc(xst[:], c)
                    nc.vector.scalar_tensor_tensor(out=yhs[dt][:rows], in0=xsc,
                                                   scalar=W["dv"][dt][:], in1=yhs[dt][:rows],
                                                   op0=OP_.mult, op1=OP_.add)
                    pdst = _rc(Pt[:], c) if rev else _fc(Pt[:], c)
                    if k == 0:
                        eng.tensor_copy(out=pdst, in_=yhs[dt][:rows])
                    else:
                        eng.tensor_tensor(out=pdst, in0=yhs[dt][:rows], in1=pdst,
                                          op=OP_.add)

        # ---- Q = mrow*P + mcol*transpose(P) ----
        Q0 = big.tile([DT0, L], f32, tag="s3b", name="Q0")
        Q1 = big.tile([DT1, L], f32, tag="s5", name="Q1")
        nc.vector.tensor_scalar_mul(Q0[:], _twh(P0[:]), mcol0[:])
        nc.vector.scalar_tensor_tensor(out=Q0[:], in0=P0[:], scalar=mrow0[:],
                                       in1=Q0[:], op0=OP_.mult, op1=OP_.add)
        nc.vector.tensor_scalar_mul(Q1[:], _twh(P1[:]), mcol1[:])
        nc.vector.scalar_tensor_tensor(out=Q1[:], in0=P1[:], scalar=mrow1[:],
                                       in1=Q1[:], op0=OP_.mult, op1=OP_.add)
        nc.sync.dma_start(oq_d[0:DT0], Q0[:])
        nc.sync.dma_start(oq_d[DT0:DIN], Q1[:])
    nc.compile()
    return nc


# ---------------------------------------------------------------- pass 2
def build_nc2():
    nc = bacc.Bacc("TRN2", target_bir_lowering=False, debug=False, num_devices=8)
    din = {}

    def I(name, shape):
        din[name] = nc.dram_tensor(name, shape, f32, kind="ExternalInput")

    I("ym", [DIN, L]); I("xin", [COUT, L]); I("zin", [DIN, L])
    I("OPm", [DIN, COUT]); I("OPB", [DIN, COUT])
    I("PW1", [COUT, HID]); I("g1", [HID, 1]); I("bb1", [HID, 1])
    I("CDW", [HID, 9]); I("g2", [HID, 1]); I("bb2", [HID, 1])
    I("PW2", [HID, COUT]); I("g3", [COUT, 1]); I("bb3", [COUT, 1])
    I("fw", [COUT, 1]); I("fb", [COUT, 1])
    out_d = nc.dram_tensor("o", [COUT, L], f32, kind="ExternalOutput")

    ctx = contextlib.ExitStack()
    with tile.TileContext(nc) as tc, ctx:
        const = ctx.enter_context(tc.tile_pool(name="const", bufs=1))
        big = ctx.enter_context(tc.tile_pool(name="big", bufs=1))
        work = ctx.enter_context(tc.tile_pool(name="work", bufs=2))
        psM = ctx.enter_context(tc.tile_pool(name="psM", bufs=2, space="PSUM"))

        def load2(name, rows, cols):
            t0 = const.tile([DT0, cols], f32, tag=name + "0", name=name + "0")
            t1 = const.tile([DT1, cols], f32, tag=name + "1", name=name + "1")
            nc.sync.dma_start(t0[:], din[name][0:DT0])
            nc.sync.dma_start(t1[:], din[name][DT0:rows])
            return t0, t1

        def load1(name, rows):
            t = const.tile([rows, 1], f32, tag=name, name=name)
            nc.sync.dma_start(t[:], din[name][:])
            return t

        OP0, OP1 = load2("OPm", DIN, COUT)
        OPB0, OPB1 = load2("OPB", DIN, COUT)
        PW1t = const.tile([COUT, HID], f32)
        nc.sync.dma_start(PW1t[:], din["PW1"][:])
        g1c0, g1c1 = load2("g1", HID, 1)
        bb1c0, bb1c1 = load2("bb1", HID, 1)
        CDW0, CDW1 = load2("CDW", HID, 9)
        g2c0, g2c1 = load2("g2", HID, 1)
        bb2c0, bb2c1 = load2("bb2", HID, 1)
        PW20, PW21 = load2("PW2", HID, COUT)
        g3c = load1("g3", COUT); bb3c = load1("bb3", COUT)
        fwc = load1("fw", COUT); fbc = load1("fb", COUT)
        ones128 = const.tile([128, 1], f32); nc.vector.memset(ones128[:], 1.0)
        onesrow = const.tile([1, 128], f32); nc.vector.memset(onesrow[:], 1.0)
        epsc = const.tile([1, 1], f32); nc.vector.memset(epsc[:], EPS)

        ym0 = big.tile([DT0, L], f32, tag="s0")
        ym1 = big.tile([DT1, L], f32, tag="s1")
        nc.sync.dma_start(ym0[:], din["ym"][0:DT0])
        nc.sync.dma_start(ym1[:], din["ym"][DT0:DIN])
        xres = big.tile([COUT, L], f32, tag="s2")
        nc.sync.dma_start(xres[:], din["xin"][:])

        # stats over 192 partitions + per-chunk post chain
        mean_r = big.tile([1, L], f32, tag="mean")
        ms_r = big.tile([1, L], f32, tag="ms")
        for (s, w) in MM:
            ps = psM.tile([128, MMC], f32, tag="mm", name="pso1")
            nc.tensor.matmul(ps[:1, :w], _r(ones128[:]), _r(ym0[:, s:s + w]),
                             start=True, stop=False)
            nc.tensor.matmul(ps[:1, :w], _r(ones128[:DT1]), _r(ym1[:, s:s + w]),
                             start=False, stop=True)
            nc.scalar.activation(mean_r[:, s:s + w], ps[:1, :w], AF.Copy,
                                 scale=1.0 / DIN)
            ps2 = psM.tile([128, MMC], f32, tag="mm", name="pso2")
            for i, (t, rows) in enumerate(((ym0, DT0), (ym1, DT1))):
                sq = work.tile([128, MMC], f32, tag="sqc", bufs=1)
                nc.vector.tensor_tensor(out=sq[:rows, :w], in0=t[:, s:s + w],
                                        in1=t[:, s:s + w], op=OP_.mult)
                nc.tensor.matmul(ps2[:1, :w], _r(ones128[:rows]), _r(sq[:rows, :w]),
                                 start=(i == 0), stop=(i == 1))
            nc.scalar.activation(ms_r[:, s:s + w], ps2[:1, :w], AF.Copy,
                                 scale=1.0 / DIN)

        x2 = big.tile([COUT, L], f32, tag="s3")
        for (s, w) in MM:
            mq = work.tile([1, MMC], f32, tag="mq", bufs=1)
            nc.vector.tensor_tensor(out=mq[:, :w], in0=mean_r[:, s:s + w],
                                    in1=mean_r[:, s:s + w], op=OP_.mult)
            nc.vector.tensor_tensor(out=mq[:, :w], in0=ms_r[:, s:s + w],
                                    in1=mq[:, :w], op=OP_.subtract)
            nc.scalar.activation(mq[:, :w], mq[:, :w], AF.Sqrt, bias=epsc[:])
            rsw = work.tile([1, MMC], f32, tag="rsw", bufs=1)
            nc.vector.reciprocal(rsw[:, :w], mq[:, :w])
            pm = psM.tile([128, MMC], f32, tag="mm", name="psm")
            nc.tensor.matmul(pm[:, :w], _r(onesrow[:]), _r(mean_r[:, s:s + w]),
                             start=True, stop=True)
            pr = psM.tile([128, MMC], f32, tag="mm", name="psr")
            nc.tensor.matmul(pr[:, :w], _r(onesrow[:]), _r(rsw[:, :w]),
                             start=True, stop=True)
            po = psM.tile([128, MMC], f32, tag="mm", name="pso")
            for i, (t, rows, zoff) in enumerate(((ym0, DT0, 0), (ym1, DT1, DT0))):
                yn = work.tile([128, MMC], f32, tag=f"yn{i}", name=f"yn{i}")
                nc.vector.tensor_tensor(out=yn[:rows, :w], in0=t[:, s:s + w],
                                        in1=pm[:rows, :w], op=OP_.subtract)
                nc.vector.tensor_tensor(out=yn[:rows, :w], in0=yn[:rows, :w],
                                        in1=pr[:rows, :w], op=OP_.mult)
                zc = work.tile([128, MMC], f32, tag=f"zc{i}", name=f"zc{i}")
                nc.sync.dma_start(zc[:rows, :w], din["zin"][zoff:zoff + rows, s:s + w])
                gc = work.tile([128, MMC], f32, tag=f"gc{i}", name=f"gc{i}")
                nc.scalar.activation(gc[:rows, :w], zc[:rows, :w], AF.Silu)
                nc.vector.tensor_tensor(out=yn[:rows, :w], in0=yn[:rows, :w],
                                        in1=gc[:rows, :w], op=OP_.mult)
                OPt = OP0 if i == 0 else OP1
                OPBt = OPB0 if i == 0 else OPB1
                nc.tensor.matmul(po[:COUT, :w], _r(OPt[:]), _r(yn[:rows, :w]),
                                 start=(i == 0), stop=False)
                nc.tensor.matmul(po[:COUT, :w], _r(OPBt[:]), _r(gc[:rows, :w]),
                                 start=False, stop=(i == 1))
            nc.vector.tensor_tensor(out=x2[:, s:s + w], in0=po[:COUT, :w],
                                    in1=xres[:, s:s + w], op=OP_.add)

        # ConvBlock
        t0 = big.tile([DT0, L], f32, tag="s4")
        t1 = big.tile([DT1, L], f32, tag="s5")
        for (s, w) in MM:
            for (dst, coff, rows, gc_, bc_) in ((t0, 0, DT0, g1c0, bb1c0),
                                                (t1, DT0, DT1, g1c1, bb1c1)):
                ps = psM.tile([128, MMC], f32, tag="mm", name="psp1")
                nc.tensor.matmul(ps[:rows, :w], _r(PW1t[:, coff:coff + rows]),
                                 _r(x2[:, s:s + w]), start=True, stop=True)
                nc.scalar.activation(dst[:, s:s + w], ps[:rows, :w], AF.Gelu,
                                     bias=bc_[:], scale=gc_[:])
        u0 = big.tile([DT0, L], f32, tag="s0b")
        u1 = big.tile([DT1, L], f32, tag="s1b")
        for (src, wt, rows, out, eng) in ((t0, CDW0, DT0, u0, nc.vector),
                                          (t1, CDW1, DT1, u1, nc.gpsimd)):
            pad = work.tile([128, 50, 50], f32, tag="pad", bufs=1)
            eng.memset(pad[:rows], 0.0)
            eng.tensor_copy(out=pad[:rows, 1:49, 1:49], in_=_pl3(src[:]))
            ov = _pl3(out[:])
            for j in range(9):
                dy, dx = divmod(j, 3)
                view = pad[:rows, dy:dy + 48, dx:dx + 48]
                if j == 0:
                    nc.vector.tensor_scalar_mul(ov, view, wt[:, 0:1])
                else:
                    nc.vector.scalar_tensor_tensor(out=ov, in0=view, scalar=wt[:, j:j + 1],
                                                   in1=ov, op0=OP_.mult, op1=OP_.add)
        x3 = big.tile([COUT, L], f32, tag="s2b")
        for (s, w) in MM:
            ps = psM.tile([128, MMC], f32, tag="mm", name="psp2")
            for i, (ut, rows, gc_, bc_) in enumerate(((u0, DT0, g2c0, bb2c0),
                                                      (u1, DT1, g2c1, bb2c1))):
                vc = work.tile([128, MMC], f32, tag=f"vc{i}", name=f"vc{i}")
                nc.scalar.activation(vc[:rows, :w], ut[:, s:s + w], AF.Gelu,
                                     bias=bc_[:], scale=gc_[:])
                PWt = PW20 if i == 0 else PW21
                nc.tensor.matmul(ps[:COUT, :w], _r(PWt[:]), _r(vc[:rows, :w]),
                                 start=(i == 0), stop=(i == 1))
            cbt = work.tile([128, MMC], f32, tag="cbt", bufs=1)
            nc.scalar.activation(cbt[:COUT, :w], ps[:COUT, :w], AF.Identity,
                                 bias=bb3c[:], scale=g3c[:])
            nc.vector.tensor_tensor(out=x3[:, s:s + w], in0=cbt[:COUT, :w],
                                    in1=x2[:, s:s + w], op=OP_.add)

        # final LN
        mean2 = big.tile([1, L], f32, tag="mean2")
        ms2 = big.tile([1, L], f32, tag="ms2")
        for (s, w) in MM:
            ps = psM.tile([128, MMC], f32, tag="mm", name="psf1")
            nc.tensor.matmul(ps[:1, :w], _r(ones128[:COUT]), _r(x3[:, s:s + w]),
                             start=True, stop=True)
            nc.scalar.activation(mean2[:, s:s + w], ps[:1, :w], AF.Copy,
                                 scale=1.0 / COUT)
            sq = work.tile([128, MMC], f32, tag="sqc", bufs=1)
            nc.vector.tensor_tensor(out=sq[:COUT, :w], in0=x3[:, s:s + w],
                                    in1=x3[:, s:s + w], op=OP_.mult)
            ps2 = psM.tile([128, MMC], f32, tag="mm", name="psf2")
            nc.tensor.matmul(ps2[:1, :w], _r(ones128[:COUT]), _r(sq[:COUT, :w]),
                             start=True, stop=True)
            nc.scalar.activation(ms2[:, s:s + w], ps2[:1, :w], AF.Copy,
                                 scale=1.0 / COUT)
        for (s, w) in MM:
            mq = work.tile([1, MMC], f32, tag="mq", bufs=1)
            nc.vector.tensor_tensor(out=mq[:, :w], in0=mean2[:, s:s + w],
                                    in1=mean2[:, s:s + w], op=OP_.mult)
            nc.vector.tensor_tensor(out=mq[:, :w], in0=ms2[:, s:s + w],
                                    in1=mq[:, :w], op=OP_.subtract)
            nc.scalar.activation(mq[:, :w], mq[:, :w], AF.Sqrt, bias=epsc[:])
            rsw = work.tile([1, MMC], f32, tag="rsw", bufs=1)
            nc.vector.reciprocal(rsw[:, :w], mq[:, :w])
            pm = psM.tile([128, MMC], f32, tag="mm", name="psfm")
            nc.tensor.matmul(pm[:, :w], _r(onesrow[:]), _r(mean2[:, s:s + w]),
                             start=True, stop=True)
            pr = psM.tile([128, MMC], f32, tag="mm", name="psfr")
            nc.tensor.matmul(pr[:, :w], _r(onesrow[:]), _r(rsw[:, :w]),
                             start=True, stop=True)
            oc = work.tile([128, MMC], f32, tag="oc", bufs=1)
            nc.vector.tensor_tensor(out=oc[:COUT, :w], in0=x3[:, s:s + w],
                                    in1=pm[:COUT, :w], op=OP_.subtract)
            nc.vector.tensor_tensor(out=oc[:COUT, :w], in0=oc[:COUT, :w],
                                    in1=pr[:COUT, :w], op=OP_.mult)
            nc.vector.tensor_scalar(out=oc[:COUT, :w], in0=oc[:COUT, :w],
                                    scalar1=fwc[:], scalar2=fbc[:],
                                    op0=OP_.mult, op1=OP_.add)
            nc.sync.dma_start(out_d[:, s:s + w], oc[:COUT, :w])
    nc.compile()
    return nc


_NC1, _NC2 = None, None


def _get_ncs():
    global _NC1, _NC2
    if _NC1 is None:
        _NC1 = build_nc1()
        _NC2 = build_nc2()
    return _NC1, _NC2


def prep_pass1(ip):
    W1 = (np.diag(ip["ln1_w"]) @ ip["in_proj_W"]).astype(np.float32)
    b1 = (ip["ln1_b"] @ ip["in_proj_W"] + ip["in_proj_b"]).astype(np.float32)
    A = (-np.exp(ip["A_logs"].astype(np.float64))).astype(np.float32).reshape(KDIR, DIN, NST)
    Ds = ip["Ds"].reshape(KDIR, DIN)
    sel16 = np.zeros((96, NST * 128), np.float32)
    for n in range(NST):
        for base in (0, 32, 64):
            sel16[base + n, n * 128:(n + 1) * 128] = 1.0
    col = lambda v: np.ascontiguousarray(v.reshape(-1, 1), dtype=np.float32)
    base = dict(projW=ip["proj_W"], projb=col(ip["proj_b"]), W1=W1, b1=col(b1),
                convW=np.ascontiguousarray(ip["conv_W"].reshape(DIN, 9)),
                convb=col(ip["conv_b"]), sel16=sel16)
    maps = []
    for c in range(8):
        b, plane = c // 2, c % 2
        ks = [plane, plane + 2]
        m = dict(base)
        m["xc_t"] = np.ascontiguousarray(ip["x_cat"][b].reshape(L, CIN).T)
        m["xpw"] = np.ascontiguousarray(np.stack([ip["x_proj_W"][k].T for k in ks]))
        m["dtw"] = np.ascontiguousarray(np.stack([ip["dt_W"][k].T for k in ks]))
        m["dtb"] = np.ascontiguousarray(np.stack([col(ip["dt_b"][k]) for k in ks]))
        m["acoef"] = np.ascontiguousarray(np.stack([A[k] for k in ks]))
        m["dvec"] = np.ascontiguousarray(np.stack([col(Ds[k]) for k in ks]))
        m["mrow"] = np.full((DIN, 1), 1.0 - plane, np.float32)
        m["mcol"] = np.full((DIN, 1), float(plane), np.float32)
        maps.append(m)
    return maps


def prep_pass2(ip, res1):
    OPm = (np.diag(ip["out_norm_w"]) @ ip["out_proj_W"]).astype(np.float32)
    OPB = (np.diag(ip["out_norm_b"]) @ ip["out_proj_W"]).astype(np.float32)
    col = lambda v: np.ascontiguousarray(v.reshape(-1, 1), dtype=np.float32)
    base = dict(OPm=OPm, OPB=OPB,
                PW1=np.ascontiguousarray(ip["cb_pw1_W"][:, :, 0, 0].T),
                g1=col(ip["cb_bn1_g"]), bb1=col(ip["cb_bn1_b"]),
                CDW=np.ascontiguousarray(ip["cb_dw_W"].reshape(HID, 9)),
                g2=col(ip["cb_bn2_g"]), bb2=col(ip["cb_bn2_b"]),
                PW2=np.ascontiguousarray(ip["cb_pw2_W"][:, :, 0, 0].T),
                g3=col(ip["cb_bn3_g"]), bb3=col(ip["cb_bn3_b"]),
                fw=col(ip["norm_w"]), fb=col(ip["norm_b"]))
    maps = []
    for c in range(8):
        b = c // 2
        m = dict(base)
        m["ym"] = res1[2 * b]["oq"] + res1[2 * b + 1]["oq"]
        m["xin"] = res1[2 * b]["ox"]
        m["zin"] = res1[2 * b]["oz"]
        maps.append(m)
    return maps


def kernel(**inputs):
    ip = {k: np.asarray(v, np.float32) for k, v in inputs.items()}
    nc1, nc2 = _get_ncs()
    res1 = run_bass_kernel_spmd(nc1, prep_pass1(ip), list(range(8))).results
    res2 = run_bass_kernel_spmd(nc2, prep_pass2(ip, res1), list(range(8))).results
    outs = [res2[2 * b]["o"].T.reshape(H_, W_, COUT) for b in range(B_)]
    return np.stack(outs).astype(np.float32)


# revision 17
# speedup vs baseline: 1986.9695x; 1986.9695x over previous
"""Trainium2 Bass kernel for nn_DecoderFusionBlock (VSS/Mamba decoder fusion block).

Two-pass SPMD over 8 cores (collectives unavailable under this runtime):
  pass 1: core c -> batch b=c//2, plane=c%2 (row-/col-major spatial order).
          Runs proj/LN/in_proj/dwconv/silu, then the selective scan for the
          plane's two directions (forward + reversed via reversed access
          patterns), producing the plane's merge partial Q (already rotated
          to row-major via data-driven masks), plus x (residual) and z (gate).
  host:   ym[b] = Q[2b] + Q[2b+1]  (the only cross-core reduction)
  pass 2: core c -> batch b=c//2: out-norm, gate, out_proj+residual,
          ConvBlock, final LN.
"""

import contextlib
import numpy as np

import concourse.bass as bass
import concourse.tile as tile
from concourse import bacc, mybir
from concourse.bass_utils import run_bass_kernel_spmd

f32 = mybir.dt.float32
f32r = mybir.dt.float32r
AF = mybir.ActivationFunctionType
OP_ = mybir.AluOpType

B_, H_, W_ = 4, 48, 48
L = H_ * W_
CIN, COUT = 192, 96
DIN, NST, RNK, KDIR = 192, 16, 6, 4
HID = 192
LC = 256
NCH = L // LC
NG = 4                           # n-values per scan group (4 groups of 4)
MMC = 512
EPS = 1e-5
DT0, DT1 = 128, 64
MM = [(s, min(MMC, L - s)) for s in range(0, L, MMC)]


def _fc(ap, c, lc=LC):
    return ap[:, c * lc:(c + 1) * lc]


def _rc(ap, c, lc=LC):
    hi = L - c * lc - 1
    lo = L - (c + 1) * lc - 1
    return ap[:, hi::-1] if lo < 0 else ap[:, hi:lo:-1]


def _swap_free(a):
    return bass.AP(tensor=a.tensor, offset=a.offset, ap=[a.ap[0], a.ap[2], a.ap[1]])


def _rep(a, n):
    return bass.AP(tensor=a.tensor, offset=a.offset, ap=[a.ap[0], [0, n], a.ap[1]])


def _twh(a):
    st = a.ap[1][0]
    return bass.AP(tensor=a.tensor, offset=a.offset,
                   ap=[a.ap[0], [st, 48], [48 * st, 48]])


def _pl3(a):
    st = a.ap[1][0]
    return bass.AP(tensor=a.tensor, offset=a.offset,
                   ap=[a.ap[0], [48 * st, 48], [st, 48]])


def _r(ap):
    # plain fp32 matmuls: the BIR verifier requires explicit rounding ops for
    # f32r operands, which would cost more than the 4x PE slowdown saves here
    return ap


# ---------------------------------------------------------------- pass 1
def build_nc1():
    nc = bacc.Bacc("TRN2", target_bir_lowering=False, debug=False, num_devices=8)
    din = {}

    def I(name, shape):
        din[name] = nc.dram_tensor(name, shape, f32, kind="ExternalInput")

    I("xc_t", [CIN, L]); I("projW", [CIN, COUT]); I("projb", [COUT, 1])
    I("W1", [COUT, 2 * DIN]); I("b1", [2 * DIN, 1])
    I("convW", [DIN, 9]); I("convb", [DIN, 1])
    I("xpw", [2, DIN, RNK + 2 * NST]); I("dtw", [2, RNK, DIN])
    I("dtb", [2, DIN, 1]); I("acoef", [2, DIN, NST]); I("dvec", [2, DIN, 1])
    I("sel16", [96, NST * 128]); I("mrow", [DIN, 1]); I("mcol", [DIN, 1])
    oq_d = nc.dram_tensor("oq", [DIN, L], f32, kind="ExternalOutput")
    ox_d = nc.dram_tensor("ox", [COUT, L], f32, kind="ExternalOutput")
    oz_d = nc.dram_tensor("oz", [DIN, L], f32, kind="ExternalOutput")

    ctx = contextlib.ExitStack()
    with tile.TileContext(nc) as tc, ctx:
        const = ctx.enter_context(tc.tile_pool(name="const", bufs=1))
        big = ctx.enter_context(tc.tile_pool(name="big", bufs=1))
        work = ctx.enter_context(tc.tile_pool(name="work", bufs=2))
        scn = ctx.enter_context(tc.tile_pool(name="scn", bufs=1))
        psM = ctx.enter_context(tc.tile_pool(name="psM", bufs=2, space="PSUM"))
        psB = ctx.enter_context(tc.tile_pool(name="psB", bufs=1, space="PSUM"))

        def load2(name, rows, cols):
            t0 = const.tile([DT0, cols], f32, tag=name + "0", name=name + "0")
            t1 = const.tile([DT1, cols], f32, tag=name + "1", name=name + "1")
            nc.sync.dma_start(t0[:], din[name][0:DT0])
            nc.sync.dma_start(t1[:], din[name][DT0:rows])
            return t0, t1

        projW0, projW1 = load2("projW", CIN, COUT)
        projb = const.tile([COUT, 1], f32)
        nc.sync.dma_start(projb[:], din["projb"][:])
        W1t = const.tile([COUT, 2 * DIN], f32)
        nc.sync.dma_start(W1t[:], din["W1"][:])
        b1x0 = const.tile([DT0, 1], f32); nc.sync.dma_start(b1x0[:], din["b1"][0:128])
        b1x1 = const.tile([DT1, 1], f32); nc.sync.dma_start(b1x1[:], din["b1"][128:192])
        b1z0 = const.tile([DT0, 1], f32); nc.sync.dma_start(b1z0[:], din["b1"][192:320])
        b1z1 = const.tile([DT1, 1], f32); nc.sync.dma_start(b1z1[:], din["b1"][320:384])
        convW0, convW1 = load2("convW", DIN, 9)
        convb0, convb1 = load2("convb", DIN, 1)
        sel16 = const.tile([96, NST * 128], f32)
        nc.sync.dma_start(sel16[:], din["sel16"][:])
        mrow0, mrow1 = load2("mrow", DIN, 1)
        mcol0, mcol1 = load2("mcol", DIN, 1)
        kw = []
        for k in range(2):
            xp0 = const.tile([DT0, RNK + 2 * NST], f32, name=f"xp{k}0")
            xp1 = const.tile([DT1, RNK + 2 * NST], f32, name=f"xp{k}1")
            nc.sync.dma_start(xp0[:], din["xpw"][k, 0:DT0])
            nc.sync.dma_start(xp1[:], din["xpw"][k, DT0:DIN])
            dtw = const.tile([RNK, DIN], f32, name=f"dtw{k}")
            nc.sync.dma_start(dtw[:], din["dtw"][k])
            dtb0 = const.tile([DT0, 1], f32, name=f"dtb{k}0")
            dtb1 = const.tile([DT1, 1], f32, name=f"dtb{k}1")
            nc.sync.dma_start(dtb0[:], din["dtb"][k, 0:DT0])
            nc.sync.dma_start(dtb1[:], din["dtb"][k, DT0:DIN])
            ac0 = const.tile([DT0, NST], f32, name=f"ac{k}0")
            ac1 = const.tile([DT1, NST], f32, name=f"ac{k}1")
            nc.sync.dma_start(ac0[:], din["acoef"][k, 0:DT0])
            nc.sync.dma_start(ac1[:], din["acoef"][k, DT0:DIN])
            dv0 = const.tile([DT0, 1], f32, name=f"dv{k}0")
            dv1 = const.tile([DT1, 1], f32, name=f"dv{k}1")
            nc.sync.dma_start(dv0[:], din["dvec"][k, 0:DT0])
            nc.sync.dma_start(dv1[:], din["dvec"][k, DT0:DIN])
            kw.append(dict(xp=(xp0, xp1), dtw=dtw, dtb=(dtb0, dtb1),
                           ac=(ac0, ac1), dv=(dv0, dv1)))

        ones128 = const.tile([128, 1], f32); nc.vector.memset(ones128[:], 1.0)
        onesrow = const.tile([1, 128], f32); nc.vector.memset(onesrow[:], 1.0)
        epsc = const.tile([1, 1], f32); nc.vector.memset(epsc[:], EPS)

        # ---- load + proj ----
        xc0 = big.tile([DT0, L], f32, tag="s0")
        xc1 = big.tile([DT1, L], f32, tag="s1")
        nc.sync.dma_start(xc0[:], din["xc_t"][0:DT0])
        nc.sync.dma_start(xc1[:], din["xc_t"][DT0:CIN])
        x_t = big.tile([COUT, L], f32, tag="s2")
        for (s, w) in MM:
            ps = psM.tile([128, MMC], f32, tag="mm", name="psproj")
            nc.tensor.matmul(ps[:COUT, :w], _r(projW0[:]), _r(xc0[:, s:s + w]),
                             start=True, stop=False)
            nc.tensor.matmul(ps[:COUT, :w], _r(projW1[:]), _r(xc1[:, s:s + w]),
                             start=False, stop=True)
            nc.scalar.activation(x_t[:, s:s + w], ps[:COUT, :w], AF.Identity,
                                 bias=projb[:])
        nc.sync.dma_start(ox_d[:], x_t[:])

        # ---- LN1 (over 96 channel partitions), fused stats+apply per chunk ----
        xn_t = big.tile([COUT, L], f32, tag="s0b")
        for (s, w) in MM:
            ps = psM.tile([128, MMC], f32, tag="mm", name="pss1")
            nc.tensor.matmul(ps[:1, :w], _r(ones128[:COUT]), _r(x_t[:, s:s + w]),
                             start=True, stop=True)
            mrw = work.tile([1, MMC], f32, tag="mrw", bufs=1)
            nc.scalar.activation(mrw[:, :w], ps[:1, :w], AF.Copy, scale=1.0 / COUT)
            sq = work.tile([128, MMC], f32, tag="sqc", bufs=1)
            nc.vector.tensor_tensor(out=sq[:COUT, :w], in0=x_t[:, s:s + w],
                                    in1=x_t[:, s:s + w], op=OP_.mult)
            ps2 = psM.tile([128, MMC], f32, tag="mm", name="pss2")
            nc.tensor.matmul(ps2[:1, :w], _r(ones128[:COUT]), _r(sq[:COUT, :w]),
                             start=True, stop=True)
            mq = work.tile([1, MMC], f32, tag="mq", bufs=1)
            nc.scalar.activation(mq[:, :w], ps2[:1, :w], AF.Copy, scale=1.0 / COUT)
            msq = work.tile([1, MMC], f32, tag="msq", bufs=1)
            nc.vector.tensor_tensor(out=msq[:, :w], in0=mrw[:, :w],
                                    in1=mrw[:, :w], op=OP_.mult)
            nc.vector.tensor_tensor(out=mq[:, :w], in0=mq[:, :w],
                                    in1=msq[:, :w], op=OP_.subtract)
            nc.scalar.activation(mq[:, :w], mq[:, :w], AF.Sqrt, bias=epsc[:])
            rsw = work.tile([1, MMC], f32, tag="rsw", bufs=1)
            nc.vector.reciprocal(rsw[:, :w], mq[:, :w])
            pm = psM.tile([128, MMC], f32, tag="mm", name="psbm")
            nc.tensor.matmul(pm[:, :w], _r(onesrow[:]), _r(mrw[:, :w]),
                             start=True, stop=True)
            pr = psM.tile([128, MMC], f32, tag="mm", name="psbr")
            nc.tensor.matmul(pr[:, :w], _r(onesrow[:]), _r(rsw[:, :w]),
                             start=True, stop=True)
            nc.vector.tensor_tensor(out=xn_t[:, s:s + w], in0=x_t[:, s:s + w],
                                    in1=pm[:COUT, :w], op=OP_.subtract)
            nc.vector.tensor_tensor(out=xn_t[:, s:s + w], in0=xn_t[:, s:s + w],
                                    in1=pr[:COUT, :w], op=OP_.mult)

        # ---- in_proj (x-part to xm tiles, z-part straight to DRAM) ----
        xm0 = big.tile([DT0, L], f32, tag="s3")
        xm1 = big.tile([DT1, L], f32, tag="s1b")
        for (s, w) in MM:
            for (coff, rows, bcol, dst, zoff) in (
                    (0, DT0, b1x0, xm0, None), (DT0, DT1, b1x1, xm1, None),
                    (DIN, DT0, b1z0, None, 0), (DIN + DT0, DT1, b1z1, None, DT0)):
                ps = psM.tile([128, MMC], f32, tag="mm", name="psip")
                nc.tensor.matmul(ps[:rows, :w], _r(W1t[:, coff:coff + rows]),
                                 _r(xn_t[:, s:s + w]), start=True, stop=True)
                if dst is not None:
                    nc.scalar.activation(dst[:, s:s + w], ps[:rows, :w], AF.Identity,
                                         bias=bcol[:])
                else:
                    zc = work.tile([128, MMC], f32, tag="zc", bufs=1)
                    nc.scalar.activation(zc[:rows, :w], ps[:rows, :w], AF.Identity,
                                         bias=bcol[:])
                    nc.sync.dma_start(oz_d[zoff:zoff + rows, s:s + w], zc[:rows, :w])

        # ---- depthwise conv + silu ----
        cv0 = big.tile([DT0, L], f32, tag="s2b")
        cv1 = big.tile([DT1, L], f32, tag="s4")
        for (src, wt, rows, out, eng) in ((xm0, convW0, DT0, cv0, nc.vector),
                                          (xm1, convW1, DT1, cv1, nc.gpsimd)):
            pad = work.tile([128, 50, 50], f32, tag="pad", bufs=1)
            eng.memset(pad[:rows], 0.0)
            eng.tensor_copy(out=pad[:rows, 1:49, 1:49], in_=_pl3(src[:]))
            ov = _pl3(out[:])
            onpool = eng is nc.gpsimd
            for j in range(9):
                dy, dx = divmod(j, 3)
                view = pad[:rows, dy:dy + 48, dx:dx + 48]
                if j == 0:
                    eng.tensor_scalar_mul(ov, view, wt[:, 0:1]) if eng is nc.gpsimd \
                        else nc.vector.tensor_scalar_mul(ov, view, wt[:, 0:1])
                else:
                    nc.vector.scalar_tensor_tensor(out=ov, in0=view, scalar=wt[:, j:j + 1],
                                                   in1=ov, op0=OP_.mult, op1=OP_.add)
        xs0 = big.tile([DT0, L], f32, tag="s3b")
        xs1 = big.tile([DT1, L], f32, tag="s5")
        nc.scalar.activation(xs0[:], cv0[:], AF.Silu, bias=convb0[:])
        nc.scalar.activation(xs1[:], cv1[:], AF.Silu, bias=convb1[:])
        # transposed plane, then data-driven select (both in place into xs)
        xt0 = big.tile([DT0, L], f32, tag="s2b", name="xt0")
        xt1 = big.tile([DT1, L], f32, tag="s4", name="xt1")
        nc.vector.tensor_copy(out=xt0[:], in_=_twh(xs0[:]))
        nc.gpsimd.tensor_copy(out=xt1[:], in_=_twh(xs1[:]))
        nc.vector.tensor_scalar_mul(xs0[:], xs0[:], mrow0[:])
        nc.vector.scalar_tensor_tensor(out=xs0[:], in0=xt0[:], scalar=mcol0[:],
                                       in1=xs0[:], op0=OP_.mult, op1=OP_.add)
        nc.gpsimd.tensor_scalar_mul(xs1[:], xs1[:], mrow1[:])
        xtm = big.tile([DT1, L], f32, tag="s4c", name="xtm")
        nc.gpsimd.tensor_scalar_mul(xtm[:], xt1[:], mcol1[:])
        nc.gpsimd.tensor_tensor(out=xs1[:], in0=xtm[:], in1=xs1[:], op=OP_.add)

        # ---- scan: k=0 forward, k=1 reversed ----
        P0 = big.tile([DT0, L], f32, tag="s6")
        P1 = big.tile([DT1, L], f32, tag="s7")
        for k in range(2):
            rev = (k == 1)
            W = kw[k]
            U96 = big.tile([96, L], f32, tag="u96", name=f"u96_{k}")
            for (s, w) in MM:
                ps = psM.tile([128, MMC], f32, tag="mm", name="psU")
                for (coff, ubase, m) in ((0, 0, RNK), (RNK, 32, NST),
                                         (RNK + NST, 64, NST)):
                    nc.tensor.matmul(ps[ubase:ubase + m, :w],
                                     _r(W["xp"][0][:, coff:coff + m]),
                                     _r(xs0[:, s:s + w]), start=True, stop=False)
                    nc.tensor.matmul(ps[ubase:ubase + m, :w],
                                     _r(W["xp"][1][:, coff:coff + m]),
                                     _r(xs1[:, s:s + w]), start=False, stop=True)
                for (ubase, m) in ((0, RNK), (32, NST), (64, NST)):
                    nc.vector.tensor_copy(out=U96[ubase:ubase + m, s:s + w],
                                          in_=ps[ubase:ubase + m, :w])

            hp = [[scn.tile([128, NG, 1], f32, tag=f"hp{dt}{h}", name=f"hp{dt}{h}")
                   for h in range(NST // NG)] for dt in range(2)]
            for dt in range(2):
                for h in range(NST // NG):
                    nc.vector.memset(hp[dt][h][:], 0.0)

            for c in range(NCH):
                uslice = (_rc(U96[0:RNK, :], c) if rev else _fc(U96[0:RNK, :], c))
                bsl = (_rc(U96[32:32 + NST, :], c) if rev else _fc(U96[32:32 + NST, :], c))
                csl = (_rc(U96[64:64 + NST, :], c) if rev else _fc(U96[64:64 + NST, :], c))
                # delta / dx chunks for both dtiles
                dcs, dxs = [], []
                for dt, (rows, dtbc, eng) in enumerate(((DT0, W["dtb"][0], nc.vector),
                                                        (DT1, W["dtb"][1], nc.gpsimd))):
                    ps = psM.tile([128, LC], f32, tag="mm", name="psdt")
                    nc.tensor.matmul(ps[:rows, :], _r(W["dtw"][:, dt * DT0:dt * DT0 + rows]),
                                     _r(uslice), start=True, stop=True)
                    dc = work.tile([128, LC], f32, tag=f"dc{dt}", name=f"dc{dt}", bufs=1)
                    nc.scalar.activation(dc[:rows], ps[:rows, :], AF.Exp,
                                         bias=dtbc[:])
                    nc.scalar.activation(dc[:rows], dc[:rows], AF.Ln, bias=1.0)
                    xsc = _rc((xs0 if dt == 0 else xs1)[:], c) if rev \
                        else _fc((xs0 if dt == 0 else xs1)[:], c)
                    dxc = work.tile([128, LC], f32, tag=f"dxc{dt}", name=f"dxc{dt}", bufs=1)
                    eng.tensor_tensor(out=dxc[:rows], in0=dc[:rows], in1=xsc, op=OP_.mult)
                    dcs.append(dc); dxs.append(dxc)

                yhs = []
                for h in range(NST // NG):
                    bt = psB.tile([128, NG, LC], f32, tag="bb", name="bb")
                    ct = psB.tile([128, NG, LC], f32, tag="cb", name="cb")
                    for q in range(NG):
                        n = h * NG + q
                        slb = _r(sel16[32:48, n * 128:(n + 1) * 128])
                        slc = _r(sel16[64:80, n * 128:(n + 1) * 128])
                        nc.tensor.matmul(bt[:, q, :], slb, _r(bsl), start=True, stop=True)
                        nc.tensor.matmul(ct[:, q, :], slc, _r(csl), start=True, stop=True)
                    for dt, (rows, eng) in enumerate(((DT0, nc.vector), (DT1, nc.vector))):
                        dA = scn.tile([128, NG, LC + 1], f32, tag=f"dA{dt}",
                                      name=f"dA{dt}", bufs=2)
                        dBu = scn.tile([128, NG, LC + 1], f32, tag=f"dBu{dt}",
                                       name=f"dBu{dt}")
                        Ht = scn.tile([128, NG, LC + 1], f32, tag=f"H{dt}",
                                      name=f"H{dt}")
                        nc.vector.memset(dA[:rows, :, 0:1], 0.0)
                        nc.gpsimd.tensor_copy(out=dBu[:rows, :, 0:1], in_=hp[dt][h][:rows])
                        for q in range(NG):
                            n = h * NG + q
                            nc.scalar.activation(dA[:rows, q, 1:], dcs[dt][:rows], AF.Exp,
                                                 scale=W["ac"][dt][:, n:n + 1])
                        nc.vector.tensor_tensor(out=dBu[:rows, :, 1:],
                                                in0=_rep(dxs[dt][:rows], NG),
                                                in1=bt[:rows], op=OP_.mult)
                        eng.tensor_tensor_scan(
                            out=Ht[:rows].rearrange("p a b -> p (a b)"),
                            data0=dA[:rows].rearrange("p a b -> p (a b)"),
                            data1=dBu[:rows].rearrange("p a b -> p (a b)"),
                            initial=0.0, op0=OP_.mult, op1=OP_.add)
                        nc.gpsimd.tensor_copy(out=hp[dt][h][:rows],
                                              in_=Ht[:rows, :, LC:LC + 1])
                        # G = H * C, into dA's buffer
                        nc.vector.tensor_tensor(out=dA[:rows, :, 1:],
                                                in0=Ht[:rows, :, 1:],
                                                in1=ct[:rows], op=OP_.mult)
                        if h == 0:
                            yh = scn.tile([128, LC], f32, tag=f"yh{dt}",
                                          name=f"yh{dt}")
                            nc.vector.tensor_reduce(
                                out=yh[:rows], in_=_swap_free(dA[:rows, :, 1:]),
                                axis=mybir.AxisListType.X, op=OP_.add)
                            yhs.append(yh)
                        else:
                            yh2 = scn.tile([128, LC], f32, tag=f"yh2{dt}",
                                           name=f"yh2{dt}")
                            nc.vector.tensor_reduce(
                                out=yh2[:rows], in_=_swap_free(dA[:rows, :, 1:]),
                                axis=mybir.AxisListType.X, op=OP_.add)
                            nc.gpsimd.tensor_tensor(out=yhs[dt][:rows],
                                                    in0=yhs[dt][:rows],
                                                    in1=yh2[:rows], op=OP_.add)
                # += D * xs ; accumulate into P
                for dt, (rows, Pt, xst, eng) in enumerate(
                        ((DT0, P0, xs0, nc.vector), (DT1, P1, xs1, nc.gpsimd))):
                    xsc = _rc(xst[:], c) if rev else _fc(xst[:], c)
                    dst_t = scn.tile([128, LC], f32, tag=f"dst{dt}", name=f"dst{dt}")
                    nc.gpsimd.tensor_scalar_mul(dst_t[:rows], xsc, W["dv"][dt][:])
                    nc.gpsimd.tensor_tensor(out=yhs[dt][:rows], in0=dst_t[:rows],
                                            in1=yhs[dt][:rows], op=OP_.add)
                    pdst = _rc(Pt[:], c) if rev else _fc(Pt[:], c)
                    if k == 0:
                        eng.tensor_copy(out=pdst, in_=yhs[dt][:rows])
                    else:
                        eng.tensor_tensor(out=pdst, in0=yhs[dt][:rows], in1=pdst,
                                          op=OP_.add)

        # ---- Q = mrow*P + mcol*transpose(P) ----
        Q0 = big.tile([DT0, L], f32, tag="s3b", name="Q0")
        Q1 = big.tile([DT1, L], f32, tag="s5", name="Q1")
        nc.vector.tensor_scalar_mul(Q0[:], _twh(P0[:]), mcol0[:])
        nc.vector.scalar_tensor_tensor(out=Q0[:], in0=P0[:], scalar=mrow0[:],
                                       in1=Q0[:], op0=OP_.mult, op1=OP_.add)
        nc.gpsimd.tensor_scalar_mul(Q1[:], _twh(P1[:]), mcol1[:])
        qtm = big.tile([DT1, L], f32, tag="s4c", name="qtm")
        nc.gpsimd.tensor_scalar_mul(qtm[:], P1[:], mrow1[:])
        nc.gpsimd.tensor_tensor(out=Q1[:], in0=qtm[:], in1=Q1[:], op=OP_.add)
        nc.sync.dma_start(oq_d[0:DT0], Q0[:])
        nc.sync.dma_start(oq_d[DT0:DIN], Q1[:])
    nc.compile()
    return nc


# ---------------------------------------------------------------- pass 2
def build_nc2():
    nc = bacc.Bacc("TRN2", target_bir_lowering=False, debug=False, num_devices=8)
    din = {}

    def I(name, shape):
        din[name] = nc.dram_tensor(name, shape, f32, kind="ExternalInput")

    I("ym", [DIN, L]); I("xin", [COUT, L]); I("zin", [DIN, L])
    I("OPm", [DIN, COUT]); I("OPB", [DIN, COUT])
    I("PW1", [COUT, HID]); I("g1", [HID, 1]); I("bb1", [HID, 1])
    I("CDW", [HID, 9]); I("g2", [HID, 1]); I("bb2", [HID, 1])
    I("PW2", [HID, COUT]); I("g3", [COUT, 1]); I("bb3", [COUT, 1])
    I("fw", [COUT, 1]); I("fb", [COUT, 1])
    out_d = nc.dram_tensor("o", [COUT, L], f32, kind="ExternalOutput")

    ctx = contextlib.ExitStack()
    with tile.TileContext(nc) as tc, ctx:
        const = ctx.enter_context(tc.tile_pool(name="const", bufs=1))
        big = ctx.enter_context(tc.tile_pool(name="big", bufs=1))
        work = ctx.enter_context(tc.tile_pool(name="work", bufs=2))
        psM = ctx.enter_context(tc.tile_pool(name="psM", bufs=2, space="PSUM"))

        def load2(name, rows, cols):
            t0 = const.tile([DT0, cols], f32, tag=name + "0", name=name + "0")
            t1 = const.tile([DT1, cols], f32, tag=name + "1", name=name + "1")
            nc.sync.dma_start(t0[:], din[name][0:DT0])
            nc.sync.dma_start(t1[:], din[name][DT0:rows])
            return t0, t1

        def load1(name, rows):
            t = const.tile([rows, 1], f32, tag=name, name=name)
            nc.sync.dma_start(t[:], din[name][:])
            return t

        OP0, OP1 = load2("OPm", DIN, COUT)
        OPB0, OPB1 = load2("OPB", DIN, COUT)
        PW1t = const.tile([COUT, HID], f32)
        nc.sync.dma_start(PW1t[:], din["PW1"][:])
        g1c0, g1c1 = load2("g1", HID, 1)
        bb1c0, bb1c1 = load2("bb1", HID, 1)
        CDW0, CDW1 = load2("CDW", HID, 9)
        g2c0, g2c1 = load2("g2", HID, 1)
        bb2c0, bb2c1 = load2("bb2", HID, 1)
        PW20, PW21 = load2("PW2", HID, COUT)
        g3c = load1("g3", COUT); bb3c = load1("bb3", COUT)
        fwc = load1("fw", COUT); fbc = load1("fb", COUT)
        ones128 = const.tile([128, 1], f32); nc.vector.memset(ones128[:], 1.0)
        onesrow = const.tile([1, 128], f32); nc.vector.memset(onesrow[:], 1.0)
        epsc = const.tile([1, 1], f32); nc.vector.memset(epsc[:], EPS)

        ym0 = big.tile([DT0, L], f32, tag="s0")
        ym1 = big.tile([DT1, L], f32, tag="s1")
        nc.sync.dma_start(ym0[:], din["ym"][0:DT0])
        nc.sync.dma_start(ym1[:], din["ym"][DT0:DIN])
        xres = big.tile([COUT, L], f32, tag="s2")
        nc.sync.dma_start(xres[:], din["xin"][:])

        # stats over 192 partitions + per-chunk post chain
        mean_r = big.tile([1, L], f32, tag="mean")
        ms_r = big.tile([1, L], f32, tag="ms")
        for (s, w) in MM:
            ps = psM.tile([128, MMC], f32, tag="mm", name="pso1")
            nc.tensor.matmul(ps[:1, :w], _r(ones128[:]), _r(ym0[:, s:s + w]),
                             start=True, stop=False)
            nc.tensor.matmul(ps[:1, :w], _r(ones128[:DT1]), _r(ym1[:, s:s + w]),
                             start=False, stop=True)
            nc.scalar.activation(mean_r[:, s:s + w], ps[:1, :w], AF.Copy,
                                 scale=1.0 / DIN)
            ps2 = psM.tile([128, MMC], f32, tag="mm", name="pso2")
            for i, (t, rows) in enumerate(((ym0, DT0), (ym1, DT1))):
                sq = work.tile([128, MMC], f32, tag="sqc", bufs=1)
                nc.vector.tensor_tensor(out=sq[:rows, :w], in0=t[:, s:s + w],
                                        in1=t[:, s:s + w], op=OP_.mult)
                nc.tensor.matmul(ps2[:1, :w], _r(ones128[:rows]), _r(sq[:rows, :w]),
                                 start=(i == 0), stop=(i == 1))
            nc.scalar.activation(ms_r[:, s:s + w], ps2[:1, :w], AF.Copy,
                                 scale=1.0 / DIN)

        x2 = big.tile([COUT, L], f32, tag="s3")
        for (s, w) in MM:
            mq = work.tile([1, MMC], f32, tag="mq", bufs=1)
            nc.vector.tensor_tensor(out=mq[:, :w], in0=mean_r[:, s:s + w],
                                    in1=mean_r[:, s:s + w], op=OP_.mult)
            nc.vector.tensor_tensor(out=mq[:, :w], in0=ms_r[:, s:s + w],
                                    in1=mq[:, :w], op=OP_.subtract)
            nc.scalar.activation(mq[:, :w], mq[:, :w], AF.Sqrt, bias=epsc[:])
            rsw = work.tile([1, MMC], f32, tag="rsw", bufs=1)
            nc.vector.reciprocal(rsw[:, :w], mq[:, :w])
            pm = psM.tile([128, MMC], f32, tag="mm", name="psm")
            nc.tensor.matmul(pm[:, :w], _r(onesrow[:]), _r(mean_r[:, s:s + w]),
                             start=True, stop=True)
            pr = psM.tile([128, MMC], f32, tag="mm", name="psr")
            nc.tensor.matmul(pr[:, :w], _r(onesrow[:]), _r(rsw[:, :w]),
                             start=True, stop=True)
            po = psM.tile([128, MMC], f32, tag="mm", name="pso")
            for i, (t, rows, zoff) in enumerate(((ym0, DT0, 0), (ym1, DT1, DT0))):
                yn = work.tile([128, MMC], f32, tag=f"yn{i}", name=f"yn{i}")
                nc.vector.tensor_tensor(out=yn[:rows, :w], in0=t[:, s:s + w],
                                        in1=pm[:rows, :w], op=OP_.subtract)
                nc.vector.tensor_tensor(out=yn[:rows, :w], in0=yn[:rows, :w],
                                        in1=pr[:rows, :w], op=OP_.mult)
                zc = work.tile([128, MMC], f32, tag=f"zc{i}", name=f"zc{i}")
                nc.sync.dma_start(zc[:rows, :w], din["zin"][zoff:zoff + rows, s:s + w])
                gc = work.tile([128, MMC], f32, tag=f"gc{i}", name=f"gc{i}")
                nc.scalar.activation(gc[:rows, :w], zc[:rows, :w], AF.Silu)
                nc.vector.tensor_tensor(out=yn[:rows, :w], in0=yn[:rows, :w],
                                        in1=gc[:rows, :w], op=OP_.mult)
                OPt = OP0 if i == 0 else OP1
                OPBt = OPB0 if i == 0 else OPB1
                nc.tensor.matmul(po[:COUT, :w], _r(OPt[:]), _r(yn[:rows, :w]),
                                 start=(i == 0), stop=False)
                nc.tensor.matmul(po[:COUT, :w], _r(OPBt[:]), _r(gc[:rows, :w]),
                                 start=False, stop=(i == 1))
            nc.vector.tensor_tensor(out=x2[:, s:s + w], in0=po[:COUT, :w],
                                    in1=xres[:, s:s + w], op=OP_.add)

        # ConvBlock
        t0 = big.tile([DT0, L], f32, tag="s4")
        t1 = big.tile([DT1, L], f32, tag="s5")
        for (s, w) in MM:
            for (dst, coff, rows, gc_, bc_) in ((t0, 0, DT0, g1c0, bb1c0),
                                                (t1, DT0, DT1, g1c1, bb1c1)):
                ps = psM.tile([128, MMC], f32, tag="mm", name="psp1")
                nc.tensor.matmul(ps[:rows, :w], _r(PW1t[:, coff:coff + rows]),
                                 _r(x2[:, s:s + w]), start=True, stop=True)
                nc.scalar.activation(dst[:, s:s + w], ps[:rows, :w], AF.Gelu,
                                     bias=bc_[:], scale=gc_[:])
        u0 = big.tile([DT0, L], f32, tag="s0b")
        u1 = big.tile([DT1, L], f32, tag="s1b")
        for (src, wt, rows, out, eng) in ((t0, CDW0, DT0, u0, nc.vector),
                                          (t1, CDW1, DT1, u1, nc.gpsimd)):
            pad = work.tile([128, 50, 50], f32, tag="pad", bufs=1)
            eng.memset(pad[:rows], 0.0)
            eng.tensor_copy(out=pad[:rows, 1:49, 1:49], in_=_pl3(src[:]))
            ov = _pl3(out[:])
            onpool = eng is nc.gpsimd
            for j in range(9):
                dy, dx = divmod(j, 3)
                view = pad[:rows, dy:dy + 48, dx:dx + 48]
                if j == 0:
                    eng.tensor_scalar_mul(ov, view, wt[:, 0:1]) if eng is nc.gpsimd \
                        else nc.vector.tensor_scalar_mul(ov, view, wt[:, 0:1])
                else:
                    nc.vector.scalar_tensor_tensor(out=ov, in0=view, scalar=wt[:, j:j + 1],
                                                   in1=ov, op0=OP_.mult, op1=OP_.add)
        x3 = big.tile([COUT, L], f32, tag="s2b")
        for (s, w) in MM:
            ps = psM.tile([128, MMC], f32, tag="mm", name="psp2")
            for i, (ut, rows, gc_, bc_) in enumerate(((u0, DT0, g2c0, bb2c0),
                                                      (u1, DT1, g2c1, bb2c1))):
                vc = work.tile([128, MMC], f32, tag=f"vc{i}", name=f"vc{i}")
                nc.scalar.activation(vc[:rows, :w], ut[:, s:s + w], AF.Gelu,
                                     bias=bc_[:], scale=gc_[:])
                PWt = PW20 if i == 0 else PW21
                nc.tensor.matmul(ps[:COUT, :w], _r(PWt[:]), _r(vc[:rows, :w]),
                                 start=(i == 0), stop=(i == 1))
            cbt = work.tile([128, MMC], f32, tag="cbt", bufs=1)
            nc.scalar.activation(cbt[:COUT, :w], ps[:COUT, :w], AF.Identity,
                                 bias=bb3c[:], scale=g3c[:])
            nc.vector.tensor_tensor(out=x3[:, s:s + w], in0=cbt[:COUT, :w],
                                    in1=x2[:, s:s + w], op=OP_.add)

        # final LN
        mean2 = big.tile([1, L], f32, tag="mean2")
        ms2 = big.tile([1, L], f32, tag="ms2")
        for (s, w) in MM:
            ps = psM.tile([128, MMC], f32, tag="mm", name="psf1")
            nc.tensor.matmul(ps[:1, :w], _r(ones128[:COUT]), _r(x3[:, s:s + w]),
                             start=True, stop=True)
            nc.scalar.activation(mean2[:, s:s + w], ps[:1, :w], AF.Copy,
                                 scale=1.0 / COUT)
            sq = work.tile([128, MMC], f32, tag="sqc", bufs=1)
            nc.vector.tensor_tensor(out=sq[:COUT, :w], in0=x3[:, s:s + w],
                                    in1=x3[:, s:s + w], op=OP_.mult)
            ps2 = psM.tile([128, MMC], f32, tag="mm", name="psf2")
            nc.tensor.matmul(ps2[:1, :w], _r(ones128[:COUT]), _r(sq[:COUT, :w]),
                             start=True, stop=True)
            nc.scalar.activation(ms2[:, s:s + w], ps2[:1, :w], AF.Copy,
                                 scale=1.0 / COUT)
        for (s, w) in MM:
            mq = work.tile([1, MMC], f32, tag="mq", bufs=1)
            nc.vector.tensor_tensor(out=mq[:, :w], in0=mean2[:, s:s + w],
                                    in1=mean2[:, s:s + w], op=OP_.mult)
            nc.vector.tensor_tensor(out=mq[:, :w], in0=ms2[:, s:s + w],
                                    in1=mq[:, :w], op=OP_.subtract)
            nc.scalar.activation(mq[:, :w], mq[:, :w], AF.Sqrt, bias=epsc[:])
            rsw = work.tile([1, MMC], f32, tag="rsw", bufs=1)
            nc.vector.reciprocal(rsw[:, :w], mq[:, :w])
            pm = psM.tile([128, MMC], f32, tag="mm", name="psfm")
            nc.tensor.matmul(pm[:, :w], _r(onesrow[:]), _r(mean2[:, s:s + w]),
                             start=True, stop=True)
            pr = psM.tile([128, MMC], f32, tag="mm", name="psfr")
            nc.tensor.matmul(pr[:, :w], _r(onesrow[:]), _r(rsw[:, :w]),
                             start=True, stop=True)
            oc = work.tile([128, MMC], f32, tag="oc", bufs=1)
            nc.vector.tensor_tensor(out=oc[:COUT, :w], in0=x3[:, s:s + w],
                                    in1=pm[:COUT, :w], op=OP_.subtract)
            nc.vector.tensor_tensor(out=oc[:COUT, :w], in0=oc[:COUT, :w],
                                    in1=pr[:COUT, :w], op=OP_.mult)
            nc.vector.tensor_scalar(out=oc[:COUT, :w], in0=oc[:COUT, :w],
                                    scalar1=fwc[:], scalar2=fbc[:],
                                    op0=OP_.mult, op1=OP_.add)
            nc.sync.dma_start(out_d[:, s:s + w], oc[:COUT, :w])
    nc.compile()
    return nc


_NC1, _NC2 = None, None


def _get_ncs():
    global _NC1, _NC2
    if _NC1 is None:
        _NC1 = build_nc1()
        _NC2 = build_nc2()
    return _NC1, _NC2


def prep_pass1(ip):
    W1 = (np.diag(ip["ln1_w"]) @ ip["in_proj_W"]).astype(np.float32)
    b1 = (ip["ln1_b"] @ ip["in_proj_W"] + ip["in_proj_b"]).astype(np.float32)
    A = (-np.exp(ip["A_logs"].astype(np.float64))).astype(np.float32).reshape(KDIR, DIN, NST)
    Ds = ip["Ds"].reshape(KDIR, DIN)
    sel16 = np.zeros((96, NST * 128), np.float32)
    for n in range(NST):
        for base in (0, 32, 64):
            sel16[base + n, n * 128:(n + 1) * 128] = 1.0
    col = lambda v: np.ascontiguousarray(v.reshape(-1, 1), dtype=np.float32)
    base = dict(projW=ip["proj_W"], projb=col(ip["proj_b"]), W1=W1, b1=col(b1),
                convW=np.ascontiguousarray(ip["conv_W"].reshape(DIN, 9)),
                convb=col(ip["conv_b"]), sel16=sel16)
    maps = []
    for c in range(8):
        b, plane = c // 2, c % 2
        ks = [plane, plane + 2]
        m = dict(base)
        m["xc_t"] = np.ascontiguousarray(ip["x_cat"][b].reshape(L, CIN).T)
        m["xpw"] = np.ascontiguousarray(np.stack([ip["x_proj_W"][k].T for k in ks]))
        m["dtw"] = np.ascontiguousarray(np.stack([ip["dt_W"][k].T for k in ks]))
        m["dtb"] = np.ascontiguousarray(np.stack([col(ip["dt_b"][k]) for k in ks]))
        m["acoef"] = np.ascontiguousarray(np.stack([A[k] for k in ks]))
        m["dvec"] = np.ascontiguousarray(np.stack([col(Ds[k]) for k in ks]))
        m["mrow"] = np.full((DIN, 1), 1.0 - plane, np.float32)
        m["mcol"] = np.full((DIN, 1), float(plane), np.float32)
        maps.append(m)
    return maps


def prep_pass2(ip, res1):
    OPm = (np.diag(ip["out_norm_w"]) @ ip["out_proj_W"]).astype(np.float32)
    OPB = (np.diag(ip["out_norm_b"]) @ ip["out_proj_W"]).astype(np.float32)
    col = lambda v: np.ascontiguousarray(v.reshape(-1, 1), dtype=np.float32)
    base = dict(OPm=OPm, OPB=OPB,
                PW1=np.ascontiguousarray(ip["cb_pw1_W"][:, :, 0, 0].T),
                g1=col(ip["cb_bn1_g"]), bb1=col(ip["cb_bn1_b"]),
                CDW=np.ascontiguousarray(ip["cb_dw_W"].reshape(HID, 9)),
                g2=col(ip["cb_bn2_g"]), bb2=col(ip["cb_bn2_b"]),
                PW2=np.ascontiguousarray(ip["cb_pw2_W"][:, :, 0, 0].T),
                g3=col(ip["cb_bn3_g"]), bb3=col(ip["cb_bn3_b"]),
                fw=col(ip["norm_w"]), fb=col(ip["norm_b"]))
    maps = []
    for c in range(8):
        b = c // 2
        m = dict(base)
        m["ym"] = res1[2 * b]["oq"] + res1[2 * b + 1]["oq"]
        m["xin"] = res1[2 * b]["ox"]
        m["zin"] = res1[2 * b]["oz"]
        maps.append(m)
    return maps


def kernel(**inputs):
    ip = {k: np.asarray(v, np.float32) for k, v in inputs.items()}
    nc1, nc2 = _get_ncs()
    res1 = run_bass_kernel_spmd(nc1, prep_pass1(ip), list(range(8))).results
    res2 = run_bass_kernel_spmd(nc2, prep_pass2(ip, res1), list(range(8))).results
    outs = [res2[2 * b]["o"].T.reshape(H_, W_, COUT) for b in range(B_)]
    return np.stack(outs).astype(np.float32)
